# revision 1
# baseline (speedup 1.0000x reference)
"""Involution2d Bass kernel for 8 trn2 NeuronCores.

Sharding: core = 2*b + half  (b = batch 0..3, half = group-half 0..1).
Each core computes out[b, half*128:(half+1)*128, :, :].

Math: ker = A @ x[b] + b_span  with A = w_span @ w_reduce folded on host
(rank-64 factorization folded; exact up to fp rounding).
out[c,p] = sum_kk ker[g(c),kk,p] * xpad[c, p+delta_kk]

Mapping:
 - ker-gen: PE matmuls (K=256 in 2 chunks), rows permuted kk-major (j = kk*8+g).
 - per tap kk: PE "replication" matmul (selection matrix) broadcasts the 8
   group-rows of tap kk to all 128 channel partitions (PSUM).
 - DVE tensor_tensor multiplies shifted xpad view by replicated ker (PSUM src).
 - PE identity matmuls accumulate the 49 tap products in PSUM.
"""
import numpy as np
from contextlib import ExitStack

B, C, H, W = 4, 256, 64, 64
G, K, PAD, R = 16, 7, 3, 4
HW = H * W
P = 128          # partitions / channels per core
NQ = 4           # pixel chunks
QPIX = HW // NQ  # 1024 pixels per quarter (16 image rows)
QROWS = H // NQ  # 16
JPAD = 512       # padded permuted-ker rows (392 -> 512)

_CACHE = {}


def _build_nc():
    import concourse.mybir as mybir
    import concourse.tile as tile
    from concourse import bacc

    f32 = mybir.dt.float32
    nc = bacc.Bacc("TRN2", target_bir_lowering=False, debug=False)

    xb = nc.dram_tensor("xb", (P, 2, H, W), f32, kind="ExternalInput")
    at = nc.dram_tensor("at", (P, 2, JPAD), f32, kind="ExternalInput")
    bias = nc.dram_tensor("bias", (P, 4), f32, kind="ExternalInput")
    bf16 = mybir.dt.bfloat16
    rep = nc.dram_tensor("rep", (P, 16, P), bf16, kind="ExternalInput")
    ident = nc.dram_tensor("ident", (P, P), mybir.dt.bfloat16, kind="ExternalInput")
    half_sel = nc.dram_tensor("half_sel", (P, 2), f32, kind="ExternalInput")
    out = nc.dram_tensor("out", (P, HW), f32, kind="ExternalOutput")

    with tile.TileContext(nc) as tc:
        with ExitStack() as ctx:
            const = ctx.enter_context(tc.tile_pool(name="const", bufs=1))
            ps_kg = ctx.enter_context(tc.tile_pool(name="ps_kg", bufs=1, space="PSUM"))
            ps_kerb = ctx.enter_context(tc.tile_pool(name="ps_kerb", bufs=2, space="PSUM"))
            ps_acc = ctx.enter_context(tc.tile_pool(name="ps_acc", bufs=1, space="PSUM"))
            sb_prod = ctx.enter_context(tc.tile_pool(name="sb_prod", bufs=4))
            sb_out = ctx.enter_context(tc.tile_pool(name="sb_out", bufs=2))

            x_sb = const.tile([P, 2, H, W], f32)
            at_sb = const.tile([P, 2, JPAD], f32)
            bias_sb = const.tile([P, 4], f32)
            rep_sb = const.tile([P, 16, P], bf16)
            id_sb = const.tile([P, P], bf16)
            hsel_sb = const.tile([P, 2], f32)
            ker_sb = const.tile([P, 4, HW], bf16)
            xpad7 = const.tile([P, K, H + 6, W], bf16)

            nc.sync.dma_start(x_sb[:], xb[:])
            nc.sync.dma_start(at_sb[:], at[:])
            nc.sync.dma_start(bias_sb[:], bias[:])
            nc.sync.dma_start(rep_sb[:], rep[:])
            nc.sync.dma_start(id_sb[:], ident[:])
            nc.sync.dma_start(hsel_sb[:], half_sel[:])

            # ---- xpad: zero border + our half's channels via PE select ----
            # x_half[c, :, :] = x_sb[:, half]; select via matmul with hsel?
            # Simpler: both halves' copies cost 2 ACT passes; select on host
            # instead: host sends xb with OUR half's 128 channels in slot 0.
            nc.vector.memset(xpad7[:], 0.0)
            for dj in range(K):
                s = dj - 3
                a, b = max(0, -s), min(W, W - s)
                nc.scalar.copy(
                    xpad7[:, dj, 3:3 + H, a:b],
                    x_sb[:, 0, :, a + s:b + s],
                )

            # ---- ker-gen: ker_sb[:, m, :] = (at[:, :, m-tile].T @ x) + bias ----
            for m in range(4):
                for n in range(8):
                    kg = ps_kg.tile([P, 512], f32)
                    for k in range(2):
                        nc.tensor.matmul(
                            kg[:],
                            at_sb[:, k, m * P:(m + 1) * P],
                            x_sb[:, k].rearrange("p h w -> p (h w)")[:, n * 512:(n + 1) * 512],
                            start=(k == 0), stop=(k == 1),
                        )
                    nc.scalar.add(
                        ker_sb[:, m, n * 512:(n + 1) * 512], kg[:],
                        bias_sb[:, m:m + 1],
                    )

            # ---- main loop: quarters x taps ----
            import concourse.mybir as _mb
            NT = K * K
            LOOKAHEAD = 2

            sb_kerb = ctx.enter_context(tc.tile_pool(name="sb_kerb", bufs=4))

            def emit_repl(q, kk):
                mt, tt = kk // 16, kk % 16
                kerb = ps_kerb.tile([P, QPIX], f32, tag="kerb")
                rg = 32 * ((tt % 16) // 4)
                for hh in range(2):
                    nc.tensor.matmul(
                        kerb[:, hh * 512:(hh + 1) * 512],
                        rep_sb[rg:rg + 32, tt, :],
                        ker_sb[rg:rg + 32, mt, q * QPIX + hh * 512:q * QPIX + (hh + 1) * 512],
                        start=True, stop=True,
                        tile_position=(rg, 0),
                    )
                kerbS = sb_kerb.tile([P, QPIX], bf16, tag="kerbS")
                nc.scalar.copy(kerbS[:], kerb[:])
                return kerbS

            for q in range(NQ):
                acc = ps_acc.tile([P, QPIX], f32)
                r0 = q * QROWS
                buckets = [[kk for kk in range(NT) if ((kk % 16) // 4) == r]
                           for r in range(4)]
                order = []
                while any(buckets):
                    for bkt in buckets:
                        if bkt:
                            order.append(bkt.pop(0))
                kerbs = {kk: emit_repl(q, kk) for kk in order[:LOOKAHEAD]}
                for i, kk in enumerate(order):
                    di, dj = kk // K, kk % K
                    prod = sb_prod.tile([P, QROWS, W], bf16)
                    nc.vector.tensor_tensor(
                        out=prod[:],
                        in0=xpad7[:, dj, di + r0: di + r0 + QROWS, :],
                        in1=kerbs.pop(kk)[:].rearrange("p (h w) -> p h w", w=W),
                        op=_mb.AluOpType.mult,
                    )
                    if i + LOOKAHEAD < NT:
                        nkk = order[i + LOOKAHEAD]
                        kerbs[nkk] = emit_repl(q, nkk)
                    pr = prod[:].rearrange("p h w -> p (h w)")
                    for hh in range(2):
                        nc.tensor.matmul(
                            acc[:, hh * 512:(hh + 1) * 512],
                            id_sb[:],
                            pr[:, hh * 512:(hh + 1) * 512],
                            start=(i == 0), stop=(i == NT - 1),
                        )
                o_sb = sb_out.tile([P, QPIX], f32)
                nc.scalar.copy(o_sb[:], acc[:])
                nc.sync.dma_start(out[:, q * QPIX:(q + 1) * QPIX], o_sb[:])

    nc.compile()
    return nc


def _host_inputs(x, w_reduce, w_span, b_span):
    A = (w_span.astype(np.float64) @ w_reduce.astype(np.float64)).astype(np.float32)
    import ml_dtypes as _md
    ident = np.eye(P, dtype=_md.bfloat16)
    rep = np.zeros((P, 16, P), dtype=np.float32)
    for p in range(P):
        for m in range(P):
            t = p // 8
            if p == t * 8 + m // 16:
                rep[p, t, m] = 1.0
    # rep[p, t, m] = 1 iff p == t*8 + m//16
    import ml_dtypes
    rep = np.zeros((P, 16, P), dtype=np.float32)
    for t in range(16):
        for m in range(P):
            rep[t * 8 + m // 16, t, m] = 1.0
    rep = rep.astype(ml_dtypes.bfloat16)

    in_maps = []
    for core in range(8):
        b, half = core // 2, core % 2
        # permuted fold: j = kk*8 + g  ->  A row (half*8+g)*49 + kk
        Ap = np.zeros((JPAD, C), dtype=np.float32)
        bp = np.zeros((JPAD,), dtype=np.float32)
        for kk in range(K * K):
            for g in range(8):
                j = kk * 8 + g
                src = (half * 8 + g) * (K * K) + kk
                Ap[j] = A[src]
                bp[j] = b_span[src]
        at = np.ascontiguousarray(
            Ap.T.reshape(2, P, JPAD).transpose(1, 0, 2))  # [P, 2, JPAD]
        bias = np.ascontiguousarray(bp.reshape(4, P).T)   # [P, 4]
        xh = x[b, half * P:(half + 1) * P]                # [128, H, W] our half
        xo = x[b, (1 - half) * P:(2 - half) * P]          # other half
        xb_arr = np.stack([xh, xo], axis=1)               # [P, 2, H, W]
        # ker-gen contracts over channel chunks k=0 (rows 0..127) and k=1:
        # chunk k must hold x channels k*128..k*128+127 in ORIGINAL order.
        # With xb[:,0]=our half, xb[:,1]=other: the A columns must be permuted
        # to match: columns [half*128:(half+1)*128] first, then the rest.
        colperm = np.concatenate([
            np.arange(half * P, (half + 1) * P),
            np.arange((1 - half) * P, (2 - half) * P)])
        Ap2 = Ap[:, colperm]
        at = np.ascontiguousarray(
            Ap2.T.reshape(2, P, JPAD).transpose(1, 0, 2))
        hsel = np.zeros((P, 2), dtype=np.float32)
        hsel[:, 0] = 1.0
        in_maps.append({
            "xb": np.ascontiguousarray(xb_arr, dtype=np.float32),
            "at": at.astype(np.float32),
            "bias": bias.astype(np.float32),
            "rep": rep,
            "ident": ident,
            "half_sel": hsel,
        })
    return in_maps


def kernel(x, w_reduce, w_span, b_span):
    from concourse import bass_utils
    x = np.asarray(x, dtype=np.float32)
    w_reduce = np.asarray(w_reduce, dtype=np.float32)
    w_span = np.asarray(w_span, dtype=np.float32)
    b_span = np.asarray(b_span, dtype=np.float32)

    if "nc" not in _CACHE:
        _CACHE["nc"] = _build_nc()
    nc = _CACHE["nc"]

    in_maps = _host_inputs(x, w_reduce, w_span, b_span)
    res = bass_utils.run_bass_kernel_spmd(nc, in_maps, core_ids=list(range(8)))

    out = np.empty((B, C, H, W), dtype=np.float32)
    for core in range(8):
        b, half = core // 2, core % 2
        out[b, half * P:(half + 1) * P] = res.results[core]["out"].reshape(P, H, W)
    return out



# revision 32
# speedup vs baseline: 2.5724x; 2.5724x over previous
"""Involution2d Bass kernel for 8 trn2 NeuronCores — pixel-major design.

Sharding: core = 2*b + half (b = batch 0..3, half = channel-half 0..1).
Each core computes out[b, half*128:(half+1)*128, :, :].

Layout (per core): partitions q = y*2 + xh  (y image row, xh x-half),
free dim f = c'*256 + g*32 + x'  (c' channel-in-group, g group, x' = x%32).

Math: ker = A @ x[b] + b_span with A = w_span @ w_reduce folded on host.
out[c, p] = sum_{kk} ker[g(c), kk, p] * xpad[c, p + delta_kk]

Device pipeline:
 - ker-gen TRANSPOSED: PE matmuls with lhsT = x pixel-columns (stride-32 AP)
   so PSUM comes out pixel-major [q, j=(kk,g)]; ACT copies -> kerT SBUF bf16.
   Two j-superblocks (di 0-2, di 3-6) so early taps start before ker-gen ends.
 - multiply: tensor_tensor per tap, in0 = host-preshifted xT7[di] slice
   (dj via free-dim offset), in1 = kerT row broadcast over c' (0-stride AP).
   Runs on DVE (2x mode) for most taps, Pool engine for a few (balance).
 - accumulate: PE identity matmuls into PSUM; a few DVE pair-adds reduce
   PE passes. ker-gen scratch lives inside the acc PSUM tile (start=True
   of the first accumulation pass resets it).
"""
import numpy as np
from contextlib import ExitStack

B, C, H, W = 4, 256, 64, 64
G, K, PAD, R = 16, 7, 3, 4
HW = H * W
P = 128
NJ = 392          # 49 taps * 8 groups per core
XW = 38           # stored x-window per partition (32 + 2*3 halo)
NKK = K * K

# --- tuning knobs -----------------------------------------------------------
POOL_TAPS = {(0, 3), (1, 3), (2, 3), (3, 3), (4, 3), (5, 3), (6, 3),
             (1, 5), (3, 5), (5, 5), (0, 5)}
PAIR_TAPS = {2: [(1, 2)], 4: [(1, 2)], 6: [(1, 2)]}
# ker-gen j-superblocks: (di range start, di range end)
KBLOCKS = [(0, 2), (2, 7)]
ACC_SPLIT = 8     # id-acc matmuls per stream (PSUM bank = 512 fp32)

_CACHE = {}


def _build_nc():
    import concourse.mybir as mybir
    import concourse.tile as tile
    from concourse import bacc

    f32 = mybir.dt.float32
    bf16 = mybir.dt.bfloat16
    mult = mybir.AluOpType.mult
    addop = mybir.AluOpType.add

    nc = bacc.Bacc("TRN2", target_bir_lowering=False, debug=False)

    n_pairs = sum(len(v) for v in PAIR_TAPS.values())

    x_ch = nc.dram_tensor("x_ch", (P, 2, HW), bf16, kind="ExternalInput")
    at = nc.dram_tensor("at", (P, 2, NJ), bf16, kind="ExternalInput")
    xT7 = nc.dram_tensor("xT7", (P, K, 16, 8, XW), bf16, kind="ExternalInput")
    bias = nc.dram_tensor("bias", (1, NJ), bf16, kind="ExternalInput")
    ones = nc.dram_tensor("ones", (1, P), bf16, kind="ExternalInput")
    idm = nc.dram_tensor("idm", (P, P), bf16, kind="ExternalInput")
    out = nc.dram_tensor("out", (P, HW), bf16, kind="ExternalOutput")

    n_streams = NKK - n_pairs

    with tile.TileContext(nc) as tc:
        with ExitStack() as ctx:
            const = ctx.enter_context(tc.tile_pool(name="const", bufs=1))
            prodp = ctx.enter_context(tc.tile_pool(name="prod", bufs=8))
            outp = ctx.enter_context(tc.tile_pool(name="outp", bufs=4))

            x_sb = const.tile([P, 2, HW], bf16)
            at_sb = const.tile([P, 2, NJ], bf16)
            bias_sb = const.tile([1, NJ], bf16)
            ones_sb = const.tile([1, P], bf16)
            idm_sb = const.tile([P, P], bf16)
            xT7_t = [const.tile([P, 16, 8, XW], bf16, name=f"xT7_{i}")
                     for i in range(K)]
            # one kerT tile per superblock: [q, kk_in_block, g, x']
            kerT_t = [const.tile([P, 7 * (d1 - d0), 8, 32], bf16, name=f"kerT_{i}")
                      for i, (d0, d1) in enumerate(KBLOCKS)]

            wtile = const.tile([P, 512], bf16)

            nc.sync.dma_start(at_sb[:], at[:])
            nc.sync.dma_start(bias_sb[:], bias[:])
            nc.sync.dma_start(ones_sb[:], ones[:])
            nc.sync.dma_start(x_sb[:, 0], x_ch[:, 0])
            nc.sync.dma_start(x_sb[:, 1], x_ch[:, 1])
            nc.sync.dma_start(xT7_t[0][:], xT7[:, 0])
            nc.sync.dma_start(idm_sb[:], idm[:])
            for di in range(1, K):
                nc.sync.dma_start(xT7_t[di][:], xT7[:, di])

            # ---- ker-gen (transposed, per superblock) ----
            # Dedicated PSUM scratch pool (closed before the accumulator
            # pool opens so the space is reused): 2 rotating tiles of
            # [P, 4, 512] fp32 (4 banks each), 4 pixel-columns per block,
            # k-phases split so k=0 matmuls run during the x k=1 DMA.
            with ExitStack() as kctx:
                ps_ker = kctx.enter_context(
                    tc.tile_pool(name="ps_ker", bufs=2, space="PSUM"))

                # PE p-state warmup on junk data while input DMAs stream in
                nc.vector.memset(wtile[:], 0.0)
                wps = ps_ker.tile([P, 4, 512], f32, tag="scr")
                for i in range(5):
                    nc.tensor.matmul(
                        wps[:, 0, :], wtile[:, 0:128], wtile[:],
                        start=(i == 0), stop=(i == 4),
                    )

                for bi, (d0, d1) in enumerate(KBLOCKS):
                    j0, j1 = 56 * d0, 56 * d1
                    J = j1 - j0
                    for m0 in range(0, 32, 4):
                        scr = ps_ker.tile([P, 4, 512], f32, tag="scr")
                        for i in range(4):
                            nc.tensor.matmul(
                                scr[:, i, 0:J], x_sb[:, 0, m0 + i::32],
                                at_sb[:, 0, j0:j1], start=True, stop=False)
                        for i in range(4):
                            nc.tensor.matmul(
                                scr[:, i, 0:J], x_sb[:, 1, m0 + i::32],
                                at_sb[:, 1, j0:j1], start=False, stop=False)
                        for i in range(4):
                            nc.tensor.matmul(
                                scr[:, i, 0:J], ones_sb[0:1, :],
                                bias_sb[0:1, j0:j1], start=False, stop=True)
                        # block-0 copies alternate onto the (still idle) DVE
                        # to halve the copy chain that gates the first taps
                        ceng = nc.vector if (bi == 0 and (m0 // 4) % 2 == 1) \
                            else nc.scalar
                        csrc = scr[:, :, 0:J].rearrange(
                            "p m (kk g) -> p kk g m", g=8)
                        cdst = kerT_t[bi][:, :, :, m0:m0 + 4]
                        if ceng is nc.scalar:
                            ceng.copy(cdst, csrc)
                        else:
                            ceng.tensor_copy(out=cdst, in_=csrc)

            ps_acc = ctx.enter_context(
                tc.tile_pool(name="ps_acc", bufs=1, space="PSUM"))
            acc = ps_acc.tile([P, HW], f32)

            def kerT_view(di, dj):
                for bi, (d0, d1) in enumerate(KBLOCKS):
                    if d0 <= di < d1:
                        return kerT_t[bi][:, (di - d0) * 7 + dj, :, :]
                raise AssertionError

            # ---- multiply + accumulate ----
            sidx = [0]

            def emit_acc(t):
                tf = t[:].rearrange("p a b c -> p (a b c)")
                n = ACC_SPLIT
                w = HW // n
                last = sidx[0] == n_streams - 1
                for c in range(n):
                    nc.tensor.matmul(
                        acc[:, c * w:(c + 1) * w],
                        idm_sb[:],
                        tf[:, c * w:(c + 1) * w],
                        start=(sidx[0] == 0), stop=last,
                    )
                    if last and c % 2 == 1:
                        # drain pairs of chunks as their accumulation closes
                        lo = (c - 1) * w
                        o_sb = outp.tile([P, 2 * w], bf16, tag="o")
                        if c % 4 == 1:
                            nc.scalar.copy(o_sb[:], acc[:, lo:lo + 2 * w])
                        else:
                            nc.vector.tensor_copy(
                                out=o_sb[:], in_=acc[:, lo:lo + 2 * w])
                        nc.sync.dma_start(out[:, lo:lo + 2 * w], o_sb[:])
                sidx[0] += 1

            def emit_mult(di, dj, eng):
                prod = prodp.tile([P, 16, 8, 32], bf16, tag="prod")
                eng.tensor_tensor(
                    out=prod[:],
                    in0=xT7_t[di][:, :, :, dj:dj + 32],
                    in1=kerT_view(di, dj).unsqueeze(1).broadcast_to(
                        (P, 16, 8, 32)),
                    op=mult,
                )
                return prod

            for di in range(K):
                pool_djs = [d for d in range(K) if (di, d) in POOL_TAPS]
                dve_djs = [d for d in range(K) if (di, d) not in POOL_TAPS]
                pairs = PAIR_TAPS.get(di, [])
                paired_djs = {d for p in pairs for d in p}
                # emit pool mults first so Pool engine races ahead
                pool_prods = [emit_mult(di, d, nc.gpsimd) for d in pool_djs]
                pend = {}
                dve_streams = []
                for dj in dve_djs:
                    prod = emit_mult(di, dj, nc.vector)
                    if dj in paired_djs:
                        pend[dj] = prod
                        pr = next(p for p in pairs if dj in p)
                        if all(d in pend for d in pr):
                            a, b = (pend[d] for d in pr)
                            psum = prodp.tile([P, 16, 8, 32], bf16, tag="prod")
                            nc.vector.tensor_tensor(
                                out=psum[:], in0=a[:], in1=b[:], op=addop)
                            dve_streams.append(psum)
                    else:
                        dve_streams.append(prod)
                # interleave pool streams at ~40% and ~85% positions
                order = list(dve_streams)
                if pool_prods:
                    order.insert(max(1, (len(order) * 2) // 5), pool_prods[0])
                for p in pool_prods[1:]:
                    order.insert(max(1, len(order) - 1), p)
                for t in order:
                    emit_acc(t)

            assert sidx[0] == n_streams

            # (drain is emitted inline with the final stream in emit_acc)

    nc.compile()
    return nc


def _host_inputs(x, w_reduce, w_span, b_span):
    import ml_dtypes
    bf = ml_dtypes.bfloat16
    A = (w_span.astype(np.float64) @ w_reduce.astype(np.float64)).astype(np.float32)
    ident = np.eye(P, dtype=np.float32).astype(bf)
    ones = np.ones((1, P), dtype=np.float32).astype(bf)

    xpad = np.zeros((B, C, H + 2 * PAD, W + 2 * PAD), dtype=np.float32)
    xpad[:, :, PAD:PAD + H, PAD:PAD + W] = x
    xpad_bf = xpad.astype(bf)

    in_maps = []
    for core in range(8):
        b, half = core // 2, core % 2
        gl = np.arange(8)
        kkv = np.arange(NKK)
        rows = ((half * 8 + gl[None, :]) * NKK + kkv[:, None]).reshape(-1)
        Ap = A[rows]                                 # [392, 256]
        bp = b_span[rows].astype(np.float32)
        at = np.ascontiguousarray(
            Ap.T.reshape(2, P, NJ).transpose(1, 0, 2)).astype(bf)

        x_ch = np.ascontiguousarray(
            x[b].reshape(2, P, HW).transpose(1, 0, 2)).astype(bf)

        xh = xpad_bf[b, half * P:(half + 1) * P]     # [128, 70, 70]
        xg = xh.reshape(8, 16, H + 6, W + 6)
        y = np.arange(H)
        di = np.arange(K)
        rowidx = (y[None, :] + di[:, None])          # [7, 64]
        xrows = xg[:, :, rowidx, :]                  # [g, c', di, y, 70]
        w0 = xrows[..., 0:XW]
        w1 = xrows[..., 32:32 + XW]
        xw2 = np.stack([w0, w1], axis=4)             # [g,c',di,y,2,38]
        xT7 = np.ascontiguousarray(xw2.transpose(3, 4, 2, 1, 0, 5).reshape(
            H * 2, K, 16, 8, XW))

        in_maps.append({
            "x_ch": x_ch,
            "at": at,
            "xT7": xT7,
            "bias": bp.reshape(1, NJ).astype(bf),
            "ones": ones,
            "idm": ident,
        })
    return in_maps


def _assemble(results):
    out = np.empty((B, C, H, W), dtype=np.float32)
    for core in range(8):
        b, half = core // 2, core % 2
        r = results[core].astype(np.float32).reshape(H, 2, 16, 8, 32)
        r = r.transpose(3, 2, 0, 1, 4).reshape(8 * 16, H, W)
        out[b, half * P:(half + 1) * P] = r
    return out


def kernel(x, w_reduce, w_span, b_span):
    from concourse import bass_utils
    x = np.asarray(x, dtype=np.float32)
    w_reduce = np.asarray(w_reduce, dtype=np.float32)
    w_span = np.asarray(w_span, dtype=np.float32)
    b_span = np.asarray(b_span, dtype=np.float32)

    if "nc" not in _CACHE:
        _CACHE["nc"] = _build_nc()
    nc = _CACHE["nc"]

    in_maps = _host_inputs(x, w_reduce, w_span, b_span)
    res = bass_utils.run_bass_kernel_spmd(nc, in_maps, core_ids=list(range(8)))
    return _assemble([res.results[c]["out"] for c in range(8)])


# revision 48
# speedup vs baseline: 2.6456x; 1.0285x over previous
"""Involution2d Bass kernel for 8 trn2 NeuronCores — pixel-major design.

Sharding: core = 2*b + half (b = batch 0..3, half = channel-half 0..1).
Each core computes out[b, half*128:(half+1)*128, :, :].

Layout (per core): partitions q = y*2 + xh  (y image row, xh x-half),
free dim f = c'*256 + g*32 + x'  (c' channel-in-group, g group, x' = x%32).

Math: ker = A @ x[b] + b_span with A = w_span @ w_reduce folded on host.
out[c, p] = sum_{kk} ker[g(c), kk, p] * xpad[c, p + delta_kk]

Device pipeline:
 - ker-gen TRANSPOSED: PE matmuls with lhsT = x pixel-columns (stride-32 AP)
   so PSUM comes out pixel-major [q, j=(kk,g)]; ACT copies -> kerT SBUF bf16.
   Two j-superblocks (di 0-2, di 3-6) so early taps start before ker-gen ends.
 - multiply: tensor_tensor per tap, in0 = host-preshifted xT7[di] slice
   (dj via free-dim offset), in1 = kerT row broadcast over c' (0-stride AP).
   Runs on DVE (2x mode) for most taps, Pool engine for a few (balance).
 - accumulate: PE identity matmuls into PSUM; a few DVE pair-adds reduce
   PE passes. ker-gen scratch lives inside the acc PSUM tile (start=True
   of the first accumulation pass resets it).
"""
import numpy as np
from contextlib import ExitStack

B, C, H, W = 4, 256, 64, 64
G, K, PAD, R = 16, 7, 3, 4
HW = H * W
P = 128
NJ = 392          # 49 taps * 8 groups per core
XW = 38           # stored x-window per partition (32 + 2*3 halo)
NKK = K * K

# --- tuning knobs -----------------------------------------------------------
POOL_TAPS = {(0, 3), (1, 3), (2, 3), (3, 3), (4, 3), (5, 3), (6, 3),
             (1, 5), (3, 5), (5, 5), (0, 5)}
PAIR_TAPS = {4: [(1, 2)]}
# ker-gen j-superblocks: (di range start, di range end)
KBLOCKS = [(0, 2), (2, 7)]
ACC_SPLIT = 8     # id-acc matmuls per stream (PSUM bank = 512 fp32)

_CACHE = {}


def _build_nc():
    import concourse.mybir as mybir
    import concourse.tile as tile
    from concourse import bacc

    f32 = mybir.dt.float32
    bf16 = mybir.dt.bfloat16
    mult = mybir.AluOpType.mult
    addop = mybir.AluOpType.add

    nc = bacc.Bacc("TRN2", target_bir_lowering=False, debug=False)

    n_pairs = sum(len(v) for v in PAIR_TAPS.values())

    x_ch = nc.dram_tensor("x_ch", (P, 2, HW), bf16, kind="ExternalInput")
    at = nc.dram_tensor("at", (P, 2, NJ), bf16, kind="ExternalInput")
    xT7 = nc.dram_tensor("xT7", (P, K, 16, 8, XW), bf16, kind="ExternalInput")
    bias = nc.dram_tensor("bias", (1, NJ), bf16, kind="ExternalInput")
    ones = nc.dram_tensor("ones", (1, P), bf16, kind="ExternalInput")
    idm = nc.dram_tensor("idm", (P, P), bf16, kind="ExternalInput")
    out = nc.dram_tensor("out", (P, HW), bf16, kind="ExternalOutput")

    n_streams = NKK - n_pairs

    with tile.TileContext(nc) as tc:
        with ExitStack() as ctx:
            const = ctx.enter_context(tc.tile_pool(name="const", bufs=1))
            prodp = ctx.enter_context(tc.tile_pool(name="prod", bufs=9))
            outp = ctx.enter_context(tc.tile_pool(name="outp", bufs=4))

            x_sb = const.tile([P, 2, HW], bf16)
            at_sb = const.tile([P, 2, NJ], bf16)
            bias_sb = const.tile([1, NJ], bf16)
            ones_sb = const.tile([1, P], bf16)
            idm_sb = const.tile([P, P], bf16)
            xT7_t = [const.tile([P, 16, 8, XW], bf16, name=f"xT7_{i}")
                     for i in range(K)]
            # kerT per superblock, split into x'-half tiles so taps can
            # start after only half the pixel-column copies have landed.
            kerT_t = [[const.tile([P, 7 * (d1 - d0), 8, 16], bf16,
                                  name=f"kerT_{i}_{h}") for h in range(2)]
                      for i, (d0, d1) in enumerate(KBLOCKS)]

            wtile = const.tile([P, 512], bf16)

            nc.sync.dma_start(at_sb[:], at[:])
            nc.sync.dma_start(bias_sb[:], bias[:])
            nc.sync.dma_start(ones_sb[:], ones[:])
            nc.sync.dma_start(x_sb[:, 0], x_ch[:, 0])
            nc.sync.dma_start(x_sb[:, 1], x_ch[:, 1])
            nc.sync.dma_start(xT7_t[0][:], xT7[:, 0])
            nc.sync.dma_start(idm_sb[:], idm[:])
            for di in range(1, K):
                nc.sync.dma_start(xT7_t[di][:], xT7[:, di])

            # ---- ker-gen (transposed, per superblock) ----
            # Dedicated PSUM scratch pool (closed before the accumulator
            # pool opens so the space is reused): 2 rotating tiles of
            # [P, 4, 512] fp32 (4 banks each), 4 pixel-columns per block,
            # k-phases split so k=0 matmuls run during the x k=1 DMA.
            with ExitStack() as kctx:
                ps_ker = kctx.enter_context(
                    tc.tile_pool(name="ps_ker", bufs=2, space="PSUM"))

                # PE p-state warmup on junk data while input DMAs stream in
                nc.vector.memset(wtile[:], 0.0)
                wps = ps_ker.tile([P, 4, 512], f32, tag="scr")
                for i in range(5):
                    nc.tensor.matmul(
                        wps[:, 0, :], wtile[:, 0:128], wtile[:],
                        start=(i == 0), stop=(i == 4),
                    )

                for bi, (d0, d1) in enumerate(KBLOCKS):
                    j0, j1 = 56 * d0, 56 * d1
                    J = j1 - j0
                    for m0 in range(0, 32, 4):
                        scr = ps_ker.tile([P, 4, 512], f32, tag="scr")
                        for i in range(4):
                            nc.tensor.matmul(
                                scr[:, i, 0:J], x_sb[:, 0, m0 + i::32],
                                at_sb[:, 0, j0:j1], start=True, stop=False)
                        for i in range(4):
                            nc.tensor.matmul(
                                scr[:, i, 0:J], x_sb[:, 1, m0 + i::32],
                                at_sb[:, 1, j0:j1], start=False, stop=False)
                        for i in range(4):
                            nc.tensor.matmul(
                                scr[:, i, 0:J], ones_sb[0:1, :],
                                bias_sb[0:1, j0:j1], start=False, stop=True)
                        nc.scalar.copy(
                            kerT_t[bi][m0 // 16][:, :, :,
                                                 m0 % 16:m0 % 16 + 4],
                            scr[:, :, 0:J].rearrange(
                                "p m (kk g) -> p kk g m", g=8),
                        )

            ps_acc = ctx.enter_context(
                tc.tile_pool(name="ps_acc", bufs=1, space="PSUM"))
            acc = ps_acc.tile([P, HW], f32)

            def kerT_view(di, dj, h):
                for bi, (d0, d1) in enumerate(KBLOCKS):
                    if d0 <= di < d1:
                        kk = (di - d0) * 7 + dj
                        return kerT_t[bi][h][:, kk, :, :]
                raise AssertionError

            # ---- multiply + accumulate ----
            sidx = [0]

            def emit_acc(t):
                tf = t[:].rearrange("p a b c -> p (a b c)")
                n = ACC_SPLIT
                w = HW // n
                last = sidx[0] == n_streams - 1
                for c in range(n):
                    nc.tensor.matmul(
                        acc[:, c * w:(c + 1) * w],
                        idm_sb[:],
                        tf[:, c * w:(c + 1) * w],
                        start=(sidx[0] == 0), stop=last,
                    )
                    if last and c % 2 == 1:
                        # drain pairs of chunks as their accumulation closes
                        lo = (c - 1) * w
                        o_sb = outp.tile([P, 2 * w], bf16, tag="o")
                        if c % 4 == 1:
                            nc.scalar.copy(o_sb[:], acc[:, lo:lo + 2 * w])
                        else:
                            nc.vector.tensor_copy(
                                out=o_sb[:], in_=acc[:, lo:lo + 2 * w])
                        nc.sync.dma_start(out[:, lo:lo + 2 * w], o_sb[:])
                sidx[0] += 1

            def emit_half(di, dj, eng, prod, h):
                lo = 16 * h
                eng.tensor_tensor(
                    out=prod[:, :, :, lo:lo + 16],
                    in0=xT7_t[di][:, :, :, dj + lo:dj + lo + 16],
                    in1=kerT_view(di, dj, h).unsqueeze(1).broadcast_to(
                        (P, 16, 8, 16)),
                    op=mult,
                )

            def emit_mult(di, dj, eng):
                prod = prodp.tile([P, 16, 8, 32], bf16, tag="prod")
                emit_half(di, dj, eng, prod, 0)
                emit_half(di, dj, eng, prod, 1)
                return prod

            # Emit mults in di-major order per engine; schedule the PE
            # accumulation passes globally by estimated prod-ready time so
            # the in-order PE never camps on a not-yet-ready (slow Pool)
            # stream.
            DVE_T, POOL_T, ADD_T = 2.26, 8.26, 2.26
            streams = []          # (est_ready, seq, prod)
            seq = [0]

            def note(t_est, prod):
                streams.append((t_est, seq[0], prod))
                seq[0] += 1

            tp = [POOL_T]
            tv = [DVE_T]
            for di in range(K):
                pool_djs = [d for d in range(K) if (di, d) in POOL_TAPS]
                dve_djs = [d for d in range(K) if (di, d) not in POOL_TAPS]
                pairs = PAIR_TAPS.get(di, [])
                paired_djs = {d for p in pairs for d in p}
                for dj in pool_djs:
                    prod = emit_mult(di, dj, nc.gpsimd)
                    note(tp[0], prod)
                    tp[0] += POOL_T
                # lo halves of every tap first, then hi halves: the hi-half
                # kerT copies land later, so this avoids DVE stalls at each
                # ker-gen block boundary.
                dprods = {dj: prodp.tile([P, 16, 8, 32], bf16, tag="prod",
                                         name=f"pr_{di}_{dj}")
                          for dj in dve_djs}
                for dj in dve_djs:
                    emit_half(di, dj, nc.vector, dprods[dj], 0)
                pend = {}
                for dj in dve_djs:
                    emit_half(di, dj, nc.vector, dprods[dj], 1)
                    prod = dprods[dj]
                    if dj in paired_djs:
                        pend[dj] = prod
                        tv[0] += DVE_T
                        pr = next(p for p in pairs if dj in p)
                        if all(d in pend for d in pr):
                            a, b = (pend[d] for d in pr)
                            psum = prodp.tile([P, 16, 8, 32], bf16, tag="prod")
                            nc.vector.tensor_tensor(
                                out=psum[:], in0=a[:], in1=b[:], op=addop)
                            tv[0] += ADD_T
                            note(tv[0], psum)
                    else:
                        note(tv[0], prod)
                        tv[0] += DVE_T
            for _, _, t in sorted(streams, key=lambda s: (s[0], s[1])):
                emit_acc(t)

            assert sidx[0] == n_streams

            # (drain is emitted inline with the final stream in emit_acc)

    nc.compile()
    return nc


def _host_inputs(x, w_reduce, w_span, b_span):
    import ml_dtypes
    bf = ml_dtypes.bfloat16
    A = (w_span.astype(np.float64) @ w_reduce.astype(np.float64)).astype(np.float32)
    ident = np.eye(P, dtype=np.float32).astype(bf)
    ones = np.ones((1, P), dtype=np.float32).astype(bf)

    xpad = np.zeros((B, C, H + 2 * PAD, W + 2 * PAD), dtype=np.float32)
    xpad[:, :, PAD:PAD + H, PAD:PAD + W] = x
    xpad_bf = xpad.astype(bf)

    in_maps = []
    for core in range(8):
        b, half = core // 2, core % 2
        gl = np.arange(8)
        kkv = np.arange(NKK)
        rows = ((half * 8 + gl[None, :]) * NKK + kkv[:, None]).reshape(-1)
        Ap = A[rows]                                 # [392, 256]
        bp = b_span[rows].astype(np.float32)
        at = np.ascontiguousarray(
            Ap.T.reshape(2, P, NJ).transpose(1, 0, 2)).astype(bf)

        x_ch = np.ascontiguousarray(
            x[b].reshape(2, P, HW).transpose(1, 0, 2)).astype(bf)

        xh = xpad_bf[b, half * P:(half + 1) * P]     # [128, 70, 70]
        xg = xh.reshape(8, 16, H + 6, W + 6)
        y = np.arange(H)
        di = np.arange(K)
        rowidx = (y[None, :] + di[:, None])          # [7, 64]
        xrows = xg[:, :, rowidx, :]                  # [g, c', di, y, 70]
        w0 = xrows[..., 0:XW]
        w1 = xrows[..., 32:32 + XW]
        xw2 = np.stack([w0, w1], axis=4)             # [g,c',di,y,2,38]
        xT7 = np.ascontiguousarray(xw2.transpose(3, 4, 2, 1, 0, 5).reshape(
            H * 2, K, 16, 8, XW))

        in_maps.append({
            "x_ch": x_ch,
            "at": at,
            "xT7": xT7,
            "bias": bp.reshape(1, NJ).astype(bf),
            "ones": ones,
            "idm": ident,
        })
    return in_maps


def _assemble(results):
    out = np.empty((B, C, H, W), dtype=np.float32)
    for core in range(8):
        b, half = core // 2, core % 2
        r = results[core].astype(np.float32).reshape(H, 2, 16, 8, 32)
        r = r.transpose(3, 2, 0, 1, 4).reshape(8 * 16, H, W)
        out[b, half * P:(half + 1) * P] = r
    return out


def kernel(x, w_reduce, w_span, b_span):
    from concourse import bass_utils
    x = np.asarray(x, dtype=np.float32)
    w_reduce = np.asarray(w_reduce, dtype=np.float32)
    w_span = np.asarray(w_span, dtype=np.float32)
    b_span = np.asarray(b_span, dtype=np.float32)

    if "nc" not in _CACHE:
        _CACHE["nc"] = _build_nc()
    nc = _CACHE["nc"]

    in_maps = _host_inputs(x, w_reduce, w_span, b_span)
    res = bass_utils.run_bass_kernel_spmd(nc, in_maps, core_ids=list(range(8)))
    return _assemble([res.results[c]["out"] for c in range(8)])
